# revision 1
# baseline (speedup 1.0000x reference)
import numpy as np
import jax
import jax.numpy as jnp

# ---- hardcoded problem constants (nn_Autoformer_19542101197528) ----
D_MODEL = 64
PRED_LEN = 144
L_DEC = 432
MA_K = 25
DAY_SEQ_LEN = 7
INIT_LEN = 144
C_IN = 3
TOP_K = 4
N_CORES = 8
BS, NODES = 4, 128
BN = BS * NODES            # 512
BSH = BN // N_CORES        # 64 sequences per core

_F = L_DEC // 2 + 1        # 217 rfft bins

# ---- host-precomputed DFT constants (replicated to all cores) ----
_l = np.arange(L_DEC)
_f = np.arange(_F)
_ANG = 2.0 * np.pi * np.outer(_l, _f) / L_DEC          # [432, 217]
_DFT_C = np.cos(_ANG).astype(np.float32)
_DFT_S = np.sin(_ANG).astype(np.float32)
_WGT = np.full((_F,), 2.0, np.float32)
_WGT[0] = 1.0
_WGT[-1] = 1.0
# irfft: x[l] = (1/L) sum_f wgt_f * (Re[f] cos(ang) - Im[f] sin(ang))
_IDFT_C = (_DFT_C * _WGT[None, :] / L_DEC).astype(np.float32)
_IDFT_S = (_DFT_S * _WGT[None, :] / L_DEC).astype(np.float32)


def _conv2d(x, w, b, pad, dil=(1, 1)):
    out = jax.lax.conv_general_dilated(
        x, w, (1, 1), [(pad[0], pad[0]), (pad[1], pad[1])],
        rhs_dilation=dil, dimension_numbers=('NCHW', 'OIHW', 'NCHW'))
    return out + b[None, :, None, None]


def _mean_value(q, k):
    """q,k: [B, 432, 64] -> mean over d of circular autocorr of q against k, [B, 432]."""
    Cc, Cs = jnp.asarray(_DFT_C), jnp.asarray(_DFT_S)
    B = q.shape[0]
    qt = jnp.transpose(q, (0, 2, 1)).reshape(B * D_MODEL, L_DEC)  # [B*64, 432]
    kt = jnp.transpose(k, (0, 2, 1)).reshape(B * D_MODEL, L_DEC)
    qr, qi = qt @ Cc, -(qt @ Cs)                                  # [B*64, 217]
    kr, ki = kt @ Cc, -(kt @ Cs)
    sr = jnp.mean((qr * kr + qi * ki).reshape(B, D_MODEL, _F), axis=1)
    si = jnp.mean((qi * kr - qr * ki).reshape(B, D_MODEL, _F), axis=1)
    mv = jnp.einsum('bf,lf->bl', sr, jnp.asarray(_IDFT_C)) - \
         jnp.einsum('bf,lf->bl', si, jnp.asarray(_IDFT_S))
    return mv


def _topk_weights_idx(mv):
    """mv: [B,432] local shard. Global top-k delays + per-seq softmax weights."""
    gm = jax.lax.pmean(jnp.mean(mv, axis=0), 'b')                 # [432] global mean
    _, idx = jax.lax.top_k(gm, TOP_K)                             # [4] int32, desc
    vals = mv[:, idx]                                             # [B, 4]
    e = jnp.exp(vals - jnp.max(vals, axis=1, keepdims=True))
    w = e / jnp.sum(e, axis=1, keepdims=True)
    return idx, w


def _agg(v, w, idx):
    """v: [B,432,64], w: [B,4], idx: [4] dynamic delays -> [B,432,64]."""
    B = v.shape[0]
    vv = jnp.concatenate([v, v], axis=1)                          # [B,864,64]
    out = w[:, 0][:, None, None] * jax.lax.dynamic_slice(vv, (0, idx[0], 0), (B, L_DEC, D_MODEL))
    for kk in range(1, TOP_K):
        out = out + w[:, kk][:, None, None] * \
            jax.lax.dynamic_slice(vv, (0, idx[kk], 0), (B, L_DEC, D_MODEL))
    return out


def _attn_core(q, k, v, wo, bo):
    mv = _mean_value(q, k)
    idx, w = _topk_weights_idx(mv)
    return _agg(v, w, idx) @ wo.T + bo


def _series_decomp(x):
    pad = (MA_K - 1) // 2
    left = jnp.repeat(x[:, :1, :], pad, axis=1)
    right = jnp.repeat(x[:, -1:, :], pad, axis=1)
    xp = jnp.concatenate([left, x, right], axis=1)                # [B, 456, 64]
    cs = jnp.cumsum(xp, axis=1)
    cs = jnp.concatenate([jnp.zeros_like(cs[:, :1]), cs], axis=1)
    mean = (cs[:, MA_K:] - cs[:, :-MA_K]) / MA_K
    return x - mean, mean


def _my_layernorm(x, g, b):
    mu = x.mean(-1, keepdims=True)
    var = ((x - mu) ** 2).mean(-1, keepdims=True)
    xh = (x - mu) / jnp.sqrt(var + 1e-5) * g + b
    return xh - xh.mean(axis=1, keepdims=True)


def _gelu(x):
    return 0.5 * x * (1.0 + jax.lax.erf(x / np.float32(np.sqrt(2.0))))


def _full(day, p):
    """One core's shard: day [BSH, 7, 144, 3] -> [BSH, 144]."""
    B = day.shape[0]
    x = jnp.transpose(day.reshape(B, DAY_SEQ_LEN, INIT_LEN, C_IN), (0, 3, 1, 2))
    s1 = jnp.transpose(_conv2d(x, p['conv0_w'], p['conv0_b'], (1, 0))
                       .reshape(B, D_MODEL, -1)[..., -L_DEC:], (0, 2, 1))
    s2 = jnp.transpose(_conv2d(x, p['conv1_w'], p['conv1_b'], (1, 0), dil=(1, 2))
                       .reshape(B, D_MODEL, -1)[..., -L_DEC:], (0, 2, 1))
    s3 = jnp.transpose(_conv2d(x, p['conv2_w'], p['conv2_b'], (0, 0))
                       .reshape(B, D_MODEL, -1)[..., -L_DEC:], (0, 2, 1))
    xd, cross, trend = s1, s2, s3

    # self-attention (autocorrelation)
    q = xd @ p['sa_wq'].T + p['sa_bq']
    k = xd @ p['sa_wk'].T + p['sa_bk']
    v = xd @ p['sa_wv'].T + p['sa_bv']
    xd = xd + _attn_core(q, k, v, p['sa_wo'], p['sa_bo'])
    xd, t1 = _series_decomp(xd)

    # cross-attention
    q = xd @ p['ca_wq'].T + p['ca_bq']
    k = cross @ p['ca_wk'].T + p['ca_bk']
    v = cross @ p['ca_wv'].T + p['ca_bv']
    xd = xd + _attn_core(q, k, v, p['ca_wo'], p['ca_bo'])
    xd, t2 = _series_decomp(xd)

    # FFN
    y = _gelu(xd @ p['ff1_w'].T) @ p['ff2_w'].T
    xd, t3 = _series_decomp(xd + y)

    # trend
    tsum = jnp.transpose(t1 + t2 + t3, (0, 2, 1))                 # [B,64,432]
    ttp = jnp.concatenate([tsum[:, :, -1:], tsum, tsum[:, :, :1]], axis=2)
    rt = 0.0
    for j in range(3):
        rt = rt + jnp.einsum('bcl,c->bl', ttp[:, :, j:j + L_DEC], p['proj_w'][0, :, j])
    trend = trend + rt[:, :, None]

    xd = _my_layernorm(xd, p['ln_g'], p['ln_b'])
    dec = xd[:, -PRED_LEN:] + trend[:, -PRED_LEN:]
    dec = dec @ p['pred_w'].T + p['pred_b']                       # [B,144,1]
    return dec[:, :, 0]


_PMAP = None
_P_DEV = None
_P_KEY = None


def kernel(**inputs):
    global _PMAP, _P_DEV, _P_KEY
    day_seq = np.ascontiguousarray(np.asarray(inputs['day_seq'], np.float32))
    if _PMAP is None:
        _PMAP = jax.pmap(_full, axis_name='b', in_axes=(0, None),
                         devices=jax.devices()[:N_CORES])
    # cache replicated weights across calls (same objects -> skip re-upload)
    key = tuple(sorted((k, v.ctypes.data if isinstance(v, np.ndarray) else id(v))
                       for k, v in inputs.items() if k != 'day_seq'))
    if _P_KEY != key:
        _P_DEV = {k: jnp.asarray(np.asarray(v, np.float32))
                  for k, v in inputs.items() if k != 'day_seq'}
        _P_KEY = key
    day_sh = day_seq.reshape(N_CORES, BSH, DAY_SEQ_LEN, INIT_LEN, C_IN)
    dec = _PMAP(day_sh, _P_DEV)
    return np.asarray(dec).reshape(BS, NODES, PRED_LEN).astype(np.float32)



# revision 3
# speedup vs baseline: 1.0052x; 1.0052x over previous
import numpy as np

# ---- hardcoded problem constants (nn_Autoformer_19542101197528) ----
D_MODEL = 64
PRED_LEN = 144
L_DEC = 432
DAY_SEQ_LEN = 7
INIT_LEN = 144
C_IN = 3
N_CORES = 8
BS, NODES = 4, 128
BN = BS * NODES
BSH = BN // N_CORES        # 64 sequences per core

_STATE = {}


def _ensure_built(inputs):
    """Build Bass program + jitted PJRT runner once; rebuild consts if weights change."""
    import bass_build
    global _STATE
    if 'nc' not in _STATE:
        _STATE['nc'] = bass_build.build_nc()
    key = tuple(sorted((k, v.ctypes.data if isinstance(v, np.ndarray) else id(v))
                       for k, v in inputs.items() if k != 'day_seq'))
    if _STATE.get('ckey') != key:
        p = {k: np.asarray(v, np.float32) for k, v in inputs.items() if k != 'day_seq'}
        _STATE['consts'] = bass_build.build_consts(p)
        _STATE['ckey'] = key
    if 'runner' not in _STATE:
        _STATE['runner'] = _make_runner(_STATE['nc'])
    return _STATE


def _make_runner(nc):
    """Mirror of concourse.bass2jax.run_bass_via_pjrt with a persistent jit
    (no donation; our kernel writes every output element)."""
    import jax
    import concourse.mybir as mybir
    from concourse import bass2jax
    from jax.sharding import Mesh, PartitionSpec
    from jax.experimental.shard_map import shard_map

    bass2jax.install_neuronx_cc_hook()
    partition_name = nc.partition_id_tensor.name if nc.partition_id_tensor else None
    in_names, out_names, out_avals, zero_outs = [], [], [], []
    for alloc in nc.m.functions[0].allocations:
        if not isinstance(alloc, mybir.MemoryLocationSet):
            continue
        name = alloc.memorylocations[0].name
        if alloc.kind == "ExternalInput":
            if name != partition_name:
                in_names.append(name)
        elif alloc.kind == "ExternalOutput":
            shape = tuple(alloc.tensor_shape)
            dtype = mybir.dt.np(alloc.dtype)
            out_names.append(name)
            out_avals.append(jax.core.ShapedArray(shape, dtype))
            zero_outs.append(np.zeros(shape, dtype))
    n_params = len(in_names)
    all_names = list(in_names) + list(out_names)
    if partition_name is not None:
        all_names.append(partition_name)

    def _body(*args):
        operands = list(args)
        if partition_name is not None:
            operands.append(bass2jax.partition_id_tensor())
        outs = bass2jax._bass_exec_p.bind(
            *operands,
            out_avals=tuple(out_avals),
            in_names=tuple(all_names),
            out_names=tuple(out_names),
            lowering_input_output_aliases=(),
            sim_require_finite=True,
            sim_require_nnan=True,
            nc=nc,
        )
        return tuple(outs)

    devices = jax.devices()[:N_CORES]
    mesh = Mesh(np.asarray(devices), ("core",))
    in_specs = (PartitionSpec("core"),) * (n_params + len(out_names))
    out_specs = (PartitionSpec("core"),) * len(out_names)
    sharded = jax.jit(
        shard_map(_body, mesh=mesh, in_specs=in_specs,
                  out_specs=out_specs, check_rep=False),
        keep_unused=True,
    )
    return dict(fn=sharded, in_names=in_names, out_names=out_names,
                zero_outs=zero_outs, n_params=n_params)


def _concat_inputs(st, day_seq):
    """Build the concatenated (replicated-const) input list for the runner."""
    day = np.asarray(day_seq, np.float32).reshape(
        N_CORES, BSH, DAY_SEQ_LEN, INIT_LEN, C_IN)
    day = np.ascontiguousarray(day.transpose(0, 1, 4, 2, 3))  # [8,64,3,7,144]
    r = st['runner']
    consts = st['consts']
    args = []
    for name in r['in_names']:
        if name == 'day':
            args.append(day.reshape(N_CORES * BSH, C_IN, DAY_SEQ_LEN, INIT_LEN))
        else:
            a = np.asarray(consts[name], np.float32)
            args.append(np.broadcast_to(a, (N_CORES,) + a.shape).reshape(
                (N_CORES * a.shape[0],) + a.shape[1:]))
    for z in r['zero_outs']:
        args.append(np.zeros((N_CORES * z.shape[0],) + z.shape[1:], z.dtype))
    return args


def kernel(**inputs):
    st = _ensure_built(inputs)
    args = _concat_inputs(st, inputs['day_seq'])
    outs = st['runner']['fn'](*args)
    out = np.asarray(outs[0]).reshape(N_CORES, BSH, PRED_LEN)
    return out.reshape(BS, NODES, PRED_LEN).astype(np.float32)
